# revision 24
# baseline (speedup 1.0000x reference)
"""Cross-attention (global, batch-flattened K/V) Trainium2 kernel, v3.

Problem: emb [16, 4096, 64]; two cross-attention halves:
  out_l2u = cross(q=emb[:8],  kv=emb[8:])   -> rows 0..7
  out_u2l = cross(q=emb[8:],  kv=emb[:8])   -> rows 8..15
cross(): q/k/v proj (64->512), s = einsum('bnc,nd->bcd', q, kflat),
InstanceNorm over (CH, B*CH) plane per b, softmax over d, ctx = a @ vflat^T,
out = ctx @ Wout.

Sharding: 16 (cross, q-batch) instances, 2 per core. Cores 0-3: q from
lower half (kv = upper), cores 4-7: q from upper (kv = lower). Both
instances on a core share the same kv half. No collectives.

Design: fp16 matmul operands (fp32 PSUM accum), sT layout (s stored
[d, c]: no aT transposes; softmax denominator via ones-matmuls), embT
of the kv half resident (built with paired [128,128] transposes, two
batches' channels stacked on partitions), v projected on the fly per
n-chunk (no DRAM scratch), kf projection software-pipelined two steps
ahead of its consumers, emission interleaved (embT builds between
s-phase batches; den between ctx passes) to keep the PE dense and warm.

Per-core phases:
  Eq/Q: embT_q + q projections        S(i): sT = kflat.T @ q, per batch,
  Ekv p: embT_kv pair p                     kf projected on the fly
  N(i): InstanceNorm stats             X(i): exp in place (ACT)
  D(i): softmax denom (ones matmuls)   C: per n-chunk: v proj + ctx + out
"""

import numpy as np
import concourse.bass as bass
import concourse.mybir as mybir
import concourse.tile as tile
from concourse import bacc
from concourse.bass_utils import run_bass_kernel_spmd

dt = mybir.dt
AF = mybir.ActivationFunctionType
ALU = mybir.AluOpType
F16 = dt.float16
F32 = dt.float32

B = 8            # batches per half
N = 4096         # sequence length
C = 64           # embedding channels
CH = 512         # num_heads * C
NB = N // 128    # 32 n-blocks
NG = N // 512    # 8 n-groups
CB = CH // 128   # 4 c-blocks
D = B * CH       # 4096 flattened kv dim
DB32 = D // 128  # 32 d-blocks
EPS = 1e-5
PLANE = float(CH * D)

_nc = None


def _build():
    nc = bacc.Bacc("TRN2", target_bir_lowering=False, debug=False, num_devices=8)

    embq = nc.declare_dram_parameter("embq", [2, N, C], F16, isOutput=False)
    embkv = nc.declare_dram_parameter("embkv", [B, N, C], F16, isOutput=False)
    # weights pre-replicated on rows (64 -> 128) host-side for row packing
    Wq_d = nc.declare_dram_parameter("Wq", [128, CH], F16, isOutput=False)
    # Wk/Wv zero-padded per parity: _lo = [[W],[0]], _hi = [[0],[W]] so
    # projections run as full K=128 matmuls (no LDW row-group switches)
    Wk_lo_d = nc.declare_dram_parameter("Wk_lo", [128, CH], F16, isOutput=False)
    Wk_hi_d = nc.declare_dram_parameter("Wk_hi", [128, CH], F16, isOutput=False)
    Wv_lo_d = nc.declare_dram_parameter("Wv_lo", [128, CH], F16, isOutput=False)
    Wv_hi_d = nc.declare_dram_parameter("Wv_hi", [128, CH], F16, isOutput=False)
    # Wout rearranged host-side: [p, cb, c] = Wout[cb*128+p, c]
    Wout_d = nc.declare_dram_parameter("Wout", [128, CB, C], F16, isOutput=False)
    ident_d = nc.declare_dram_parameter("ident", [128, 128], F16, isOutput=False)
    ones_d = nc.declare_dram_parameter("ones", [128, 128], F32, isOutput=False)
    out_d = nc.declare_dram_parameter("out", [2, C, N], F16, isOutput=True)

    with tile.TileContext(nc) as tc:
        with (
            tc.tile_pool(name="const", bufs=1) as constp,
            tc.tile_pool(name="io", bufs=1) as iop,
            tc.tile_pool(name="lt", bufs=2) as ltp,
            tc.tile_pool(name="embt", bufs=1) as embtp,
            tc.tile_pool(name="big", bufs=2) as bigp,
            tc.tile_pool(name="sT", bufs=1) as sTp,
            tc.tile_pool(name="kf", bufs=6) as kfp,
            tc.tile_pool(name="ctxs", bufs=1) as ctxp,
            tc.tile_pool(name="ot", bufs=2) as otp,
            tc.tile_pool(name="small", bufs=1) as smallp,
            tc.tile_pool(name="ps", bufs=1, space="PSUM") as psp,
        ):
            # ---- constants (all fp16 direct, no conversion) ----
            ident = constp.tile([128, 128], F16, tag="ident")
            nc.sync.dma_start(ident[:], ident_d[:])

            # ---- PE warm-up: dense matmuls so HAM unthrottles early ----
            wu_ps = psp.tile([128, 512], F32, tag="pp", bufs=3)
            for w in range(120):
                nc.tensor.matmul(
                    wu_ps[:, 0:128], ident[:], ident[:],
                    start=True, stop=True)

            # ---- embT layout ----
            # embT_kv[c + 64*(db%2), db//2, n] ; embT_q[c + 64*inst, n]
            embT_kv = embtp.tile([128, B // 2, N], F16, tag="embT_kv")
            embT_q = embtp.tile([128, N], F16, tag="embT_q")

            # prefetch all emb loads up front (pair-interleaved on channels:
            # lt[p, nb, b*64+c] = src[b, nb*128+p, c])
            def load_pair(src2, nm):
                lt = ltp.tile([128, NB, 2, C], F16, tag="lt", name=nm)
                for b in range(2):
                    nc.sync.dma_start(
                        lt[:, :, b, :],
                        src2[b].rearrange("(nb p) c -> p nb c", p=128))
                return lt

            lt_q = load_pair(embq[:], "ltq")
            lt_kv = [load_pair(embkv[2 * p:2 * p + 2], f"ltkv{p}")
                     for p in range(B // 2)]

            Wq_s = constp.tile([128, CH], F16, tag="Wq")
            nc.sync.dma_start(Wq_s[:], Wq_d[:])
            Wk_p = []
            for nm, wd in (("Wk_lo", Wk_lo_d), ("Wk_hi", Wk_hi_d)):
                w = constp.tile([128, CH], F16, tag=nm, name=nm)
                nc.sync.dma_start(w[:], wd[:])
                Wk_p.append(w)
            Wv_p = []
            for nm, wd in (("Wv_lo", Wv_lo_d), ("Wv_hi", Wv_hi_d)):
                w = constp.tile([128, CH], F16, tag=nm, name=nm)
                nc.sync.dma_start(w[:], wd[:])
                Wv_p.append(w)
            Wout_s = constp.tile([128, CB, C], F16, tag="Wout")
            nc.sync.dma_start(Wout_s[:], Wout_d[:])
            ones_f = iop.tile([128, 128], F32, tag="ones_f")
            nc.sync.dma_start(ones_f[:], ones_d[:])
            ones_r = constp.tile([128, 128], dt.float32r, tag="ones_r")
            nc.vector.tensor_copy(out=ones_r[:], in_=ones_f[:])
            ones16 = constp.tile([128, 1], F16, tag="ones16")
            nc.vector.tensor_copy(out=ones16[:], in_=ones_f[:, 0:1])

            def emit_group(lt, dst, G):  # one 512-n group of transposes
                pt = psp.tile([128, 512], F16, tag="pp", bufs=3)
                for j in range(4):
                    nc.tensor.transpose(
                        pt[:, j * 128:(j + 1) * 128],
                        lt[:, G * 4 + j, :, :], ident[:])
                nc.vector.tensor_copy(
                    out=dst[:, G * 512:(G + 1) * 512], in_=pt[:])

            # ---- embT_q build fused with q projections (PE stays dense) ----
            q_sb = [bigp.tile([128, NB, CH], F16, tag="big", name=f"q{i}")
                    for i in range(2)]
            emit_group(lt_q, embT_q[:], 0)
            emit_group(lt_q, embT_q[:], 1)
            for G in range(NG):
                for nb in range(G * 4, G * 4 + 4):
                    for i in range(2):
                        r = 64 * i
                        q_ps = psp.tile([128, 512], F32, tag="pp", bufs=3)
                        nc.tensor.matmul(
                            q_ps[:],
                            embT_q[r:r + 64, nb * 128:(nb + 1) * 128],
                            Wq_s[r:r + 64, :],
                            start=True, stop=True)
                        nc.vector.tensor_copy(out=q_sb[i][:, nb, :],
                                              in_=q_ps[:])
                if G + 2 < NG:
                    emit_group(lt_q, embT_q[:], G + 2)

            sT = [sTp.tile([128, DB32, CH], F16, tag=f"sT{i}", name=f"sT{i}")
                  for i in range(2)]
            ssum = [smallp.tile([128, DB32], F32, tag=f"ssum{i}",
                                name=f"ssum{i}") for i in range(2)]
            ssq = [smallp.tile([128, DB32], F32, tag=f"ssq{i}",
                               name=f"ssq{i}") for i in range(2)]
            sqscr = ctxp.tile([128, CB, 512], F16, tag="ctxs", name="sqscr")

            # ---- phase S: sT = kflat.T @ q, kf pipelined 2 steps ahead ----
            def proj_kf(step):
                db, nb = divmod(step, NB)
                kf_ps = psp.tile([128, 512], F32, tag="pp", bufs=3)
                nc.tensor.matmul(
                    kf_ps[:],
                    embT_kv[:, db // 2, nb * 128:(nb + 1) * 128],
                    Wk_p[db % 2][:],
                    start=True, stop=True)
                kf = kfp.tile([128, 512], F16, tag="kf")
                nc.vector.tensor_copy(out=kf[:], in_=kf_ps[:])
                return kf

            def s_phase(inst, dbs, pipe, build=None):
                # pipe: dict carrying the kf lookahead across calls
                for db in dbs:
                    s_ps = [psp.tile([128, 512], F32, tag="sacc", bufs=5,
                                     name=f"sacc{inst}_{db}_{k}")
                            for k in range(4)]
                    bound = (dbs[-1] + 1) * NB - 1
                    for nb in range(NB):
                        if build is not None and db == dbs[0] and nb % 4 == 0:
                            lt, dst = build
                            if nb == 0:
                                emit_group(lt, dst, 0)
                                emit_group(lt, dst, 1)
                            g = nb // 4 + 2
                            if g < NG:
                                emit_group(lt, dst, g)
                        step = db * NB + nb
                        # project kf in bursts of 4 to amortize the K=64/128
                        # LDWEIGHTS row-group switch; stay 2-6 steps ahead
                        if pipe["next"] <= min(step + 3, bound):
                            hi = min(pipe["next"] + 3, bound)
                            while pipe["next"] <= hi:
                                pipe[pipe["next"]] = proj_kf(pipe["next"])
                                pipe["next"] += 1
                        kf = pipe.pop(step)
                        for k in range(4):
                            nc.tensor.matmul(
                                s_ps[k][:],
                                kf[:, k * 128:(k + 1) * 128],
                                q_sb[inst][:, nb, :],
                                start=(nb == 0), stop=(nb == NB - 1))
                    for k in range(4):
                        dk = db * 4 + k
                        nc.scalar.activation(
                            sT[inst][:, dk, :], s_ps[k][:], AF.Copy,
                            accum_out=ssum[inst][:, dk:dk + 1])
                    # sumsq from the fp16 copies (banks already released)
                    for k in range(4):
                        dk = db * 4 + k
                        nc.scalar.activation(
                            sqscr[:, 0, :], sT[inst][:, dk, :], AF.Square,
                            accum_out=ssq[inst][:, dk:dk + 1])

            # ---- stats / exp / den per instance ----
            stats = [smallp.tile([128, 8], F32, tag=f"stats{i}",
                                 name=f"stats{i}") for i in range(2)]
            inv_den = [smallp.tile([128, CB], F32, tag=f"invden{i}",
                                   name=f"invden{i}") for i in range(2)]

            def n_phase(inst):  # InstanceNorm stats
                red = smallp.tile([128, 2], F32, tag=f"red{inst}",
                                  name=f"red{inst}")
                nc.vector.tensor_reduce(
                    out=red[:, 0:1], in_=ssum[inst][:],
                    axis=mybir.AxisListType.X, op=ALU.add)
                nc.vector.tensor_reduce(
                    out=red[:, 1:2], in_=ssq[inst][:],
                    axis=mybir.AxisListType.X, op=ALU.add)
                red_r = smallp.tile([128, 2], dt.float32r, tag=f"redr{inst}",
                                    name=f"redr{inst}")
                nc.vector.tensor_copy(out=red_r[:], in_=red[:])
                ptr = psp.tile([128, 512], F32, tag="pp", bufs=3)
                nc.tensor.matmul(
                    ptr[:, 0:2], ones_r[:], red_r[:], start=True, stop=True)
                st = stats[inst]
                nc.scalar.activation(
                    st[:, 0:2], ptr[:, 0:2], AF.Copy, bias=0.0,
                    scale=1.0 / PLANE)
                mu, ex2 = st[:, 0:1], st[:, 1:2]
                musq, var = st[:, 2:3], st[:, 3:4]
                std, rstd, nmr = st[:, 4:5], st[:, 5:6], st[:, 6:7]
                nc.vector.tensor_tensor(out=musq, in0=mu, in1=mu, op=ALU.mult)
                nc.vector.tensor_tensor(out=var, in0=ex2, in1=musq,
                                        op=ALU.subtract)
                nc.vector.tensor_scalar_add(var, var, EPS)
                nc.scalar.activation(std, var, AF.Sqrt, bias=0.0)
                nc.vector.reciprocal(rstd, std)
                nc.vector.tensor_tensor(out=nmr, in0=mu, in1=rstd,
                                        op=ALU.mult)
                nc.scalar.mul(nmr, nmr, -1.0)

            def x_phase(inst):  # exp in place, chunked
                st = stats[inst]
                for G in range(NG):
                    nc.scalar.activation(
                        sT[inst][:, G * 4:(G + 1) * 4, :],
                        sT[inst][:, G * 4:(G + 1) * 4, :],
                        AF.Exp, bias=st[:, 6:7], scale=st[:, 5:6])

            def d_phase(inst):  # softmax denominator, dense 512-wide MMs
                den_ps = psp.tile([128, 512], F32, tag="pp", bufs=3)
                for dk in range(DB32):
                    nc.tensor.matmul(
                        den_ps[0:1, :], ones16[:], sT[inst][:, dk, :],
                        start=(dk == 0), stop=(dk == DB32 - 1))
                dr = smallp.tile([1, 512], F16, tag=f"denrow{inst}",
                                 name=f"denrow{inst}")
                nc.vector.tensor_copy(out=dr[:], in_=den_ps[0:1, :])
                # spread den[c] across partitions: K=1 matmuls per c-block
                spread = psp.tile([128, 512], F32, tag="pp", bufs=3)
                for cb in range(CB):
                    nc.tensor.matmul(
                        spread[:, cb:cb + 1],
                        dr[0:1, cb * 128:(cb + 1) * 128],
                        ones16[0:1, 0:1],
                        start=(cb == 0), stop=(cb == CB - 1))
                nc.vector.reciprocal(inv_den[inst][:], spread[:, 0:CB])

            # ---- emission: interleave embT_kv builds with s-phase(0) ----
            pipe = {"next": 0}
            for p in range(B // 2):
                s_phase(0, [2 * p, 2 * p + 1], pipe,
                        build=(lt_kv[p], embT_kv[:, p, :]))
            pipe = {"next": 0}
            s_phase(1, [0], pipe)     # dense PE work over s0's drain chain
            n_phase(0)
            x_phase(0)
            s_phase(1, list(range(1, B)), pipe)
            d_phase(0)                # dense PE work over s1's drain chain
            n_phase(1)
            x_phase(1)

            # ---- phase C: v on the fly + ctx + out; d_phase(1) spliced ----
            def vp(G, va, db):  # project v for one batch / n-chunk
                Wv_s = Wv_p[db % 2]
                for chb in range(CB):
                    v_ps = psp.tile([128, 512], F32, tag="pp", bufs=3)
                    nc.tensor.matmul(
                        v_ps[:],
                        Wv_s[:, chb * 128:(chb + 1) * 128],
                        embT_kv[:, db // 2, G * 512:(G + 1) * 512],
                        start=True, stop=True)
                    nc.vector.tensor_copy(out=va[:, db, chb, :],
                                          in_=v_ps[:])

            def ctx_pass(G, inst, va, fuse_vp):
                ctx_ps = [psp.tile([128, 512], F32, tag="sacc", bufs=5,
                                   name=f"ctx{G}_{inst}_{cb}")
                          for cb in range(CB)]
                for db in range(B):
                    if fuse_vp and db + 2 < B:
                        vp(G, va, db + 2)
                    for chb in range(CB):
                        dk = db * 4 + chb
                        for cb in range(CB):
                            nc.tensor.matmul(
                                ctx_ps[cb][:],
                                sT[inst][:, dk, cb * 128:(cb + 1) * 128],
                                va[:, db, chb, :],
                                start=(dk == 0), stop=(dk == DB32 - 1))
                ctxs = ctxp.tile([128, CB, 512], F16, tag="ctxs")
                for cb in range(CB):
                    nc.scalar.activation(
                        ctxs[:, cb, :], ctx_ps[cb][:], AF.Copy,
                        scale=inv_den[inst][:, cb:cb + 1])
                out_ps = psp.tile([128, 512], F32, tag="pp", bufs=3)
                for cb in range(CB):
                    nc.tensor.matmul(
                        out_ps[0:C, :],
                        Wout_s[:, cb, :],
                        ctxs[:, cb, :],
                        start=(cb == 0), stop=(cb == CB - 1))
                ot = otp.tile([C, 512], F16, tag="ot")
                nc.vector.tensor_copy(out=ot[:], in_=out_ps[0:C, :])
                nc.sync.dma_start(
                    out_d[inst, :, G * 512:(G + 1) * 512], ot[:])

            for G in range(NG):
                va = bigp.tile([128, B, CB, 512], F16, tag="big",
                               name=f"va{G}")
                vp(G, va, 0)
                vp(G, va, 1)
                ctx_pass(G, 0, va, fuse_vp=True)
                if G == 0:
                    d_phase(1)  # PE work while ACT finishes exp(1)
                ctx_pass(G, 1, va, fuse_vp=False)

    nc.compile()
    return nc


def _get_nc():
    global _nc
    if _nc is None:
        _nc = _build()
    return _nc


def make_in_maps(emb, Wq, Wk, Wv, Wout):
    """Per-core input dicts (8 cores). Host-side fp16 casts + replication."""
    emb16 = np.ascontiguousarray(emb, dtype=np.float16)
    Wq16 = np.concatenate([Wq, Wq], axis=0).astype(np.float16)
    z = np.zeros_like(Wk)
    Wk_lo = np.concatenate([Wk, z], axis=0).astype(np.float16)
    Wk_hi = np.concatenate([z, Wk], axis=0).astype(np.float16)
    Wv_lo = np.concatenate([Wv, z], axis=0).astype(np.float16)
    Wv_hi = np.concatenate([z, Wv], axis=0).astype(np.float16)
    Wout16 = np.ascontiguousarray(
        Wout.reshape(CB, 128, C).transpose(1, 0, 2)).astype(np.float16)
    ident = np.eye(128, dtype=np.float16)
    ones = np.ones((128, 128), np.float32)
    emb_l, emb_u = emb16[:B], emb16[B:]
    in_maps = []
    for core in range(8):
        if core < 4:
            qb, kvb = emb_l[2 * core:2 * core + 2], emb_u
        else:
            j = core - 4
            qb, kvb = emb_u[2 * j:2 * j + 2], emb_l
        in_maps.append({
            "embq": np.ascontiguousarray(qb),
            "embkv": np.ascontiguousarray(kvb),
            "Wq": Wq16, "Wk_lo": Wk_lo, "Wk_hi": Wk_hi,
            "Wv_lo": Wv_lo, "Wv_hi": Wv_hi, "Wout": Wout16,
            "ident": ident, "ones": ones,
        })
    return in_maps


def kernel(emb, Wq, Wk, Wv, Wout):
    in_maps = make_in_maps(np.asarray(emb), np.asarray(Wq), np.asarray(Wk),
                           np.asarray(Wv), np.asarray(Wout))
    res = run_bass_kernel_spmd(_get_nc(), in_maps, list(range(8))).results
    out = np.empty((2 * B, N, C), np.float32)
    for core in range(8):
        o = res[core]["out"].transpose(0, 2, 1)  # [2, C, N] -> [2, N, C]
        if core < 4:
            out[2 * core:2 * core + 2] = o
        else:
            j = core - 4
            out[B + 2 * j:B + 2 * j + 2] = o
    return out
